# revision 1
# baseline (speedup 1.0000x reference)
"""GCN layer kernel for Trainium2, batch-parallel across 8 NeuronCores.

out[b] = D^-1/2 (A[b]+I) D^-1/2 @ x[b] @ W.T + b_vec

Per-core dataflow (core c owns batch element c):
  - adj slice [N,N] is streamed from HBM exactly once in 16 row-tiles.
  - Each fp32 row-tile is cast to the matmul dtype split across the
    scalar and vector engines, both producing row-sum partials via
    accum_out in the same pass (degrees come for free).
  - The PE transposes each 128x128 block into a resident A^T buffer in
    SBUF (matmul needs the contraction index on the partition dim).
  - The degree scaling is folded into x (x' = d*x) and the output
    (out = d * (...)), so adj_norm is never materialized.
  - Aggregation runs as PSUM-accumulated matmuls aggT[f, n] += x'_j^T AT_j,
    fired as soon as their row-tile dependencies are satisfied. The +I
    self-loop is folded in as x'_u^T @ I identity matmuls on the PE.
  - Tail: out[n,o] = d[n] * (aggT^T @ W^T)[n,o] + b[o], written out on
    both HWDGE queues.
"""

import numpy as np

B, N, F = 8, 2048, 128
P = 128                # partition tile / block size
NT = N // P            # 16 row tiles
NCHUNK = 512           # moving-dim chunk for the aggregation matmul
NCH = N // NCHUNK      # 4 chunks
TPC = NCHUNK // P      # row tiles per chunk
PE_BLOCKS = 16         # blocks transposed on the PE; rest via DMA xbar
WARMUP = 48            # dummy PE ops at start to lift the HAM clock gate

_PROGRAM_CACHE = {}


def _build_program(agg_dtype_name="float16", pe_blocks=PE_BLOCKS):
    import concourse.bacc as bacc
    import concourse.bass as bass
    import concourse.tile as tile
    from concourse import mybir
    from concourse.masks import make_identity

    f32 = mybir.dt.float32
    agg_dt = getattr(mybir.dt, agg_dtype_name)
    # scalar engine casts more columns than DVE: the DVE also drains the
    # transpose PSUM (where its 16-bit 2x mode makes it much faster)
    csplit = 11 * P

    nc = bacc.Bacc(
        "TRN2",
        target_bir_lowering=False,
        debug=False,
        num_devices=B,
        # no cross-core branching: per-core data arrives via in_maps, so
        # skip the partition-id register load sequence in the preamble
        enable_partition_id=False,
    )
    x_d = nc.dram_tensor("x", [N, F], f32, kind="ExternalInput")
    a_d = nc.dram_tensor("adj", [N, N], f32, kind="ExternalInput")
    w_d = nc.dram_tensor("W", [F, F], f32, kind="ExternalInput")
    b_d = nc.dram_tensor("b", [F], f32, kind="ExternalInput")
    o_d = nc.dram_tensor("out", [N, F], f32, kind="ExternalOutput")

    with tile.TileContext(nc) as tc:
        with (
            tc.tile_pool(name="singles", bufs=1) as singles,
            tc.tile_pool(name="a_in", bufs=4) as a_in,
            tc.tile_pool(name="a_cast", bufs=3) as a_cast,
            tc.tile_pool(name="x_pool", bufs=1) as x_pool,
            tc.tile_pool(name="small", bufs=6) as small,
            tc.tile_pool(name="tp_psum", bufs=2, space="PSUM") as tp_psum,
            tc.tile_pool(name="agg_psum", bufs=1, space="PSUM") as agg_psum,
            tc.tile_pool(name="o2_psum", bufs=2, space="PSUM") as o2_psum,
        ):
            ident_t = singles.tile([P, P], agg_dt)   # transpose rhs + self-loop
            make_identity(nc, ident_t)
            ident_f = singles.tile([P, P], f32)      # W transpose rhs
            make_identity(nc, ident_f)

            # dummy PE activity during the DMA fill phase lifts the HAM
            # clock gate (PE starts at K/N=4/8 and only ramps after ~4us
            # of sustained activity)
            for w in range(WARMUP // 8):
                wtp = tp_psum.tile([P, 8, P], agg_dt, name="tp", tag="tp")
                for s in range(8):
                    nc.tensor.transpose(wtp[:, s, :], ident_t, ident_t)

            # W^T: load W [o,f], transpose once -> wt_sb [f,o]
            w_sb = singles.tile([P, P], f32)
            nc.scalar.dma_start(w_sb, w_d[:, :])
            wt_ps = o2_psum.tile([P, P], f32, tag="o2")
            nc.tensor.transpose(wt_ps, w_sb, ident_f)
            wt_sb = singles.tile([P, P], agg_dt)
            nc.scalar.copy(wt_sb, wt_ps)

            # bias broadcast across partitions: b_sb[p, o] = b[o]
            b_sb = singles.tile([P, F], f32)
            b_ap = b_d[:]
            nc.scalar.dma_start(
                b_sb, bass.AP(tensor=b_ap.tensor, offset=b_ap.offset, ap=[[0, P], *b_ap.ap])
            )

            # x tiles: x_t[p, f] = x[t*P + p, f]; per-tile loads ride the adj
            # queue (one monolithic strided load is descriptor-bound), one
            # pool slot per tile so the loads carry no false dependencies
            x_tiles = [
                x_pool.tile([P, F], f32, name=f"x_t{t}", tag=f"x{t}")
                for t in range(NT)
            ]

            xp_sb = singles.tile([P, NT, F], agg_dt)   # x' = d * x
            at_sb = singles.tile([P, NT, N], agg_dt)   # resident A^T
            aggt_sb = singles.tile([P, N], agg_dt)     # aggT = (A+I)x' transposed
            out_sb = singles.tile([P, NT, F], f32)
            d_all = singles.tile([P, NT], f32)         # d = (rowsum+1)^-1/2

            # one slot per accumulator: distinct tags keep all NCH tiles
            # simultaneously resident (they accumulate across the whole kernel)
            agg_ps = [
                agg_psum.tile([P, NCHUNK], f32, name=f"agg_ps{i}", tag=f"agg{i}")
                for i in range(NCH)
            ]

            # aggregation ops per chunk: 16 regular j-steps + TPC identity
            # (self-loop) steps; each ready at a known row-tile time
            agg_plan = [[] for _ in range(NCH)]
            for i in range(NCH):
                ready_i = TPC * i + TPC - 1  # chunk's AT columns complete
                for j in range(NT):
                    agg_plan[i].append((max(ready_i, j), "j", j))
                for u in range(TPC * i, TPC * (i + 1)):
                    agg_plan[i].append((max(ready_i, u), "ident", u))
            agg_emitted = [0] * NCH
            agg_pending = []  # ready steps carried across tiles (burst cap)
            AGG_CAP = 7

            def emit_agg(i, kind, idx):
                total = len(agg_plan[i])
                first = agg_emitted[i] == 0
                agg_emitted[i] += 1
                last = agg_emitted[i] == total
                if kind == "j":
                    rhs = at_sb[:, idx, NCHUNK * i : NCHUNK * (i + 1)]
                    out_ap = agg_ps[i]
                else:
                    # self-loop: aggT[:, u] += x'_u^T (= x'_u^T @ I)
                    rhs = ident_t
                    off = P * (idx - TPC * i)
                    out_ap = agg_ps[i][:, off : off + P]
                nc.tensor.matmul(
                    out_ap,
                    xp_sb[:, idx, :],
                    rhs,
                    start=first,
                    stop=last,
                )

            def fire_agg_steps(t):
                for i in range(NCH):
                    for ready, kind, idx in agg_plan[i]:
                        if ready == t:
                            agg_pending.append((i, kind, idx))
                if t == NT - 1:
                    # final flush: emit the last chunk's big block first so
                    # the single-step chunks close near-simultaneously and
                    # their drain/out2/write chains pipeline across engines
                    agg_pending.sort(key=lambda s: -s[0])
                    budget = len(agg_pending)
                else:
                    budget = AGG_CAP
                for _ in range(min(budget, len(agg_pending))):
                    emit_agg(*agg_pending.pop(0))

            for t in range(NT):
                a_t = a_in.tile([P, N], f32)
                nc.sync.dma_start(a_t, a_d[P * t : P * (t + 1), :])
                nc.sync.dma_start(x_tiles[t], x_d[P * t : P * (t + 1), :])

                # cast + row-sum partials, split scalar/DVE; the last tile
                # splits evenly to shorten the tail-entry d chain
                cs = csplit if t < NT - 1 else N // 2
                a_c = a_cast.tile([P, N], agg_dt)
                rs_a = small.tile([P, 1], f32)
                nc.scalar.activation(
                    a_c[:, :cs],
                    a_t[:, :cs],
                    mybir.ActivationFunctionType.Copy,
                    accum_out=rs_a,
                )
                rs_b = small.tile([P, 1], f32)
                nc.vector.tensor_scalar(
                    a_c[:, cs:],
                    a_t[:, cs:],
                    1.0,
                    None,
                    op0=mybir.AluOpType.mult,
                    op1=mybir.AluOpType.add,  # accum reduce op
                    accum_out=rs_b,
                )

                # d_t = (rs_a + rs_b + 1)^-1/2  (+1 = self loop)
                # the chain hops engines but only its throughput matters:
                # gpsimd and the sqrt/recip slots are far from saturated
                rs = small.tile([P, 1], f32)
                nc.gpsimd.tensor_add(rs, rs_a, rs_b)
                sq = small.tile([P, 1], f32)
                nc.scalar.activation(
                    sq, rs, mybir.ActivationFunctionType.Sqrt, bias=1.0, scale=1.0
                )
                nc.vector.reciprocal(d_all[:, t : t + 1], sq)

                # x'_t = d_t * x_t (per-partition scale)
                nc.vector.tensor_scalar_mul(
                    xp_sb[:, t, :], x_tiles[t], d_all[:, t : t + 1]
                )

                # transpose blocks j < pe_blocks on the PE (groups of 8 per
                # PSUM bank), the rest via the DMA crossbar
                for g0 in range(0, pe_blocks, 8):
                    gn = min(8, pe_blocks - g0)
                    tp = tp_psum.tile([P, 8, P], agg_dt, name="tp", tag="tp")
                    for s in range(gn):
                        j = g0 + s
                        nc.tensor.transpose(
                            tp[:, s, :], a_c[:, P * j : P * (j + 1)], ident_t
                        )
                    dst = at_sb[:, g0 : g0 + gn, P * t : P * (t + 1)]
                    nc.vector.tensor_copy(dst, tp[:, :gn])
                if pe_blocks < NT:
                    nc.sync.dma_start_transpose(
                        at_sb[:, pe_blocks:, P * t : P * (t + 1)],
                        a_c[:, P * pe_blocks :],
                    )

                fire_agg_steps(t)

            # drain aggregation PSUM (cast to the linear-layer dtype)
            for i in range(NCH):
                dst = aggt_sb[:, NCHUNK * i : NCHUNK * (i + 1)]
                if i % 2 == 0:
                    nc.vector.tensor_copy(dst, agg_ps[i])
                else:
                    nc.scalar.copy(dst, agg_ps[i])

            for u in range(NT):
                o2 = o2_psum.tile([P, P], f32, name="o2", tag="o2")
                nc.tensor.matmul(
                    o2,
                    aggt_sb[:, P * u : P * (u + 1)],
                    wt_sb,
                    start=True,
                    stop=True,
                )
                # out = d[n] * o2 + b
                nc.vector.scalar_tensor_tensor(
                    out_sb[:, u, :],
                    o2,
                    d_all[:, u : u + 1],
                    b_sb,
                    op0=mybir.AluOpType.mult,
                    op1=mybir.AluOpType.add,
                )
                # split the descriptor-bound output writes across both
                # HWDGE queues
                eng = nc.sync if u % 2 == 0 else nc.scalar
                eng.dma_start(o_d[P * u : P * (u + 1), :], out_sb[:, u, :])

    nc.compile()
    return nc


def get_program(agg_dtype_name="float16", pe_blocks=PE_BLOCKS):
    key = (agg_dtype_name, pe_blocks)
    if key not in _PROGRAM_CACHE:
        _PROGRAM_CACHE[key] = _build_program(agg_dtype_name, pe_blocks)
    return _PROGRAM_CACHE[key]


def kernel(x, adj, W, b, _trace=False, _agg_dtype="float16", _pe_blocks=PE_BLOCKS):
    from concourse.bass_utils import run_bass_kernel_spmd

    nc = get_program(_agg_dtype, _pe_blocks)
    x = np.ascontiguousarray(np.asarray(x), dtype=np.float32)
    adj = np.ascontiguousarray(np.asarray(adj), dtype=np.float32)
    W = np.ascontiguousarray(np.asarray(W), dtype=np.float32)
    b = np.ascontiguousarray(np.asarray(b), dtype=np.float32)

    in_maps = [
        {"x": x[c], "adj": adj[c], "W": W, "b": b} for c in range(B)
    ]
    res = run_bass_kernel_spmd(
        nc, in_maps, list(range(B)), trace=_trace, trace_cores=[0] if _trace else None
    )
    out = np.stack([res.results[c]["out"] for c in range(B)], axis=0)
    if _trace:
        return out, res
    return out

